# revision 23
# baseline (speedup 1.0000x reference)
"""Multi-head attention kernel for 8 Trainium2 NeuronCores.

Problem: B=4, S=2048, D=1024, H=16 heads (d_k=64), fp32 inputs,
random 0/1 attention mask [B, S, S].

Sharding: core c -> (batch b = c//2, head-group g = c%2).  Each core
computes 8 heads of one batch: Megatron column-parallel QKV, row-parallel
output projection.  Host sums the two partial outputs per batch.

Pipeline layout (single pass, engines overlapped):
  prologue: K-proj (all keys), V-proj (all keys), Q-proj(chunk 0)
  block qc=0..3: attention(qc) | Q-proj(qc+1) | O-proj(qc)
so the ScalarE exp stream (the second-largest engine load) hides under
the PE stream instead of serializing behind a monolithic phase 1.

Bias algebra (exact):
  - bk dropped: s[k,q] += qh[q].bk is constant over k at fixed q and
    softmax over k is shift-invariant.
  - bv, bo folded to host: softmax rows sum to 1, so ctx = ctx' + bv
    and out = ctx' Wo^T + (Wo bv + bo).  Host adds the single vector.
  - bq kept on device (varies over k), scale 1/sqrt(dk) folded into Wq.

Device-side layout choices (avoids every on-device transpose):
  - host passes x^T [D, S] so projections contract D on partitions
  - projections emit qh^T / kh^T [512, S] (head dims on partitions)
  - scores are computed transposed: S^T[k, q] = kh^T.T @ qh^T
  - softmax: exp on ScalarE (no max subtraction; scores are O(5)),
    multiplicative fp16 {0,1} mask on VectorE (2x packed mode),
    denominator = ones-column appended to V in the P@V matmul
  - ctx^T[d, q] accumulates in PSUM; normalization multiplies by a
    reciprocal row broadcast across partitions via GpSimd
  - output projection consumes ctx^T directly, emits fp16 out^T partials

All matmuls keep the stock (128,128) PE tile shape — K=64 / M=65
variants measurably drop the PE instruction stream out of its fast
decode path on HW (+70ns per matmul).  vh stores 65 real columns per
head (64 v-dims + ones) in a 584-wide strip; each head's stationary
read spans 128 columns, the 63-column overlap into the next head only
lands in PSUM rows 65..127 which are never read.
"""

import numpy as np

B = 4
S = 2048
D = 1024
H = 16  # total heads
HL = 8  # heads per core
DK = 64
DH = HL * DK  # 512 local head dims
P = 128
N_CORES = 8

_compiled = None


def _build_program():
    import concourse.bacc as bacc
    import concourse.tile as tile
    from concourse import mybir

    f32 = mybir.dt.float32
    f16 = mybir.dt.float16
    AF = mybir.ActivationFunctionType

    nc = bacc.Bacc()

    # ---- DRAM I/O ----
    xqT = nc.declare_dram_parameter("xqT", [D, S], f16, isOutput=False)
    xkT = nc.declare_dram_parameter("xkT", [D, S], f16, isOutput=False)
    xvT = nc.declare_dram_parameter("xvT", [D, S], f16, isOutput=False)
    maskT = nc.declare_dram_parameter("maskT", [S, S], f16, isOutput=False)
    wqT = nc.declare_dram_parameter("wqT", [D, DH], f16, isOutput=False)
    wkT = nc.declare_dram_parameter("wkT", [D, DH], f16, isOutput=False)
    wvT = nc.declare_dram_parameter("wvT", [D, DH], f16, isOutput=False)
    woT = nc.declare_dram_parameter("woT", [DH, D], f16, isOutput=False)
    bq = nc.declare_dram_parameter("bq", [DH], f32, isOutput=False)
    outT = nc.declare_dram_parameter("outT", [D, S], f16, isOutput=True)

    KC = D // P       # 8 contraction chunks for QKV projections
    DT = DH // P      # 4 dim-tiles of qh^T/kh^T
    SC = S // 512     # 4 seq chunks of 512
    ST = S // P       # 16 seq tiles of 128
    OT = D // P       # 8 output dim tiles
    CC = DH // P      # 4 contraction chunks for O-projection
    PAIRS = HL // 2   # 4 head pairs
    VW = DK + 1       # 65 stored columns per head in vh (v-dims + ones)
    VPAD = HL * VW + P - VW  # 583 -> stationary reads stay in bounds

    with tile.TileContext(nc) as tc:
        with (
            tc.tile_pool(name="persist", bufs=1) as persist,
            tc.tile_pool(name="maskp", bufs=2) as maskp,
            tc.tile_pool(name="xs", bufs=2) as xs,
            tc.tile_pool(name="pt", bufs=3) as ptp,
            tc.tile_pool(name="small", bufs=2) as small,
            tc.tile_pool(name="outp", bufs=2) as outp,
            tc.tile_pool(name="ps", bufs=2, space="PSUM") as ps,
        ):
            qhT_sb = persist.tile([P, DT, S], f16)
            khT_sb = persist.tile([P, PAIRS, 2, S], f16)
            vh_sb = persist.tile([P, ST, VPAD + 1], f16)
            ctxT_sb = persist.tile([P, CC, S], f16)
            wq_sb = persist.tile([P, KC, DH], f16)
            wk_sb = persist.tile([P, KC, DH], f16)
            wv_sb = persist.tile([P, KC, DH], f16)
            wo_sb = persist.tile([P, CC, D], f16)
            bq_sb = persist.tile([P, DT], f32)

            def load_w(dst, src):
                nc.sync.dma_start(
                    out=dst, in_=src[:, :].rearrange("(c p) m -> p c m", p=P)
                )

            # K-proj weights + first xk chunk lead the DMA queue; the
            # rest of the bulk is staggered between the per-sc loads so
            # no xk chunk queues behind multi-MB transfers it doesn't need.
            load_w(wk_sb, wkT)
            xk_tiles = {}
            xk_tiles[0] = xs.tile([P, KC, 512], f16, name="xk_t")
            nc.sync.dma_start(
                out=xk_tiles[0], in_=xkT[:, 0:512].rearrange("(c p) j -> p c j", p=P)
            )

            # zero-init padded K layout; ones columns of vh give softmax
            # denominators.  Zero kh rows keep the full-K matmul exact.
            nc.vector.memset(khT_sb[:, :, :, :], 0.0)
            nc.vector.memset(
                vh_sb[:, :, 0 : HL * VW].rearrange(
                    "p t (h c) -> p t h c", c=VW
                )[:, :, :, DK : DK + 1],
                1.0,
            )
            # the 64-col tail is read (never written) by head 7's
            # 128-wide stationary fetch — keep it NaN-free
            nc.vector.memset(vh_sb[:, :, HL * VW :], 0.0)

            m_tiles = {}
            xq_tiles = {}

            def prefetch_mask(qc):
                m_tiles[qc] = maskp.tile([P, ST, 512], f16, name="m_sb")
                nc.sync.dma_start(
                    out=m_tiles[qc],
                    in_=maskT[:, qc * 512 : (qc + 1) * 512].rearrange(
                        "(t p) j -> p t j", p=P
                    ),
                )

            def prefetch_xq(qc):
                xq_tiles[qc] = xs.tile([P, KC, 512], f16, name="xk_t")
                nc.sync.dma_start(
                    out=xq_tiles[qc],
                    in_=xqT[:, qc * 512 : (qc + 1) * 512].rearrange(
                        "(c p) j -> p c j", p=P
                    ),
                )

            def prefetch_block(qc):
                prefetch_mask(qc)
                prefetch_xq(qc)

            # ================= prologue =================
            # K-projection: all 4 seq chunks -> khT (copies on ScalarE,
            # idle here; bk dropped exactly — softmax shift-invariance)
            for sc in range(SC):
                sl = slice(sc * 512, (sc + 1) * 512)
                xk_t = xk_tiles.pop(sc)
                if sc + 1 < SC:
                    xk_tiles[sc + 1] = xs.tile([P, KC, 512], f16, name="xk_t")
                    nc.sync.dma_start(
                        out=xk_tiles[sc + 1],
                        in_=xkT[:, (sc + 1) * 512 : (sc + 2) * 512].rearrange(
                            "(c p) j -> p c j", p=P
                        ),
                    )
                for half in range(2):
                    psk = ps.tile([P, 1024], f32, name=f"psk{sc}_{half}", tag="sps", bufs=3)
                    for sub in range(2):
                        dt_ = 2 * half + sub
                        wslice = slice(dt_ * P, (dt_ + 1) * P)
                        hsl = slice(sub * 512, sub * 512 + 512)
                        for kc in range(KC):
                            nc.tensor.matmul(
                                psk[:, hsl],
                                lhsT=wk_sb[:, kc, wslice],
                                rhs=xk_t[:, kc, :],
                                start=(kc == 0),
                                stop=(kc == KC - 1),
                            )
                    for sub in range(2):
                        dt_ = 2 * half + sub
                        src = psk[:, sub * 512 : sub * 512 + 512]
                        nc.scalar.copy(khT_sb[0:DK, dt_, 0, sl], src[0:DK, :])
                        nc.scalar.copy(khT_sb[DK : 2 * DK, dt_, 1, sl], src[DK : 2 * DK, :])
                # staggered bulk DMA: each group rides behind the xk
                # chunk it must not delay
                if sc == 0:
                    load_w(wv_sb, wvT)
                elif sc == 1:
                    load_w(wq_sb, wqT)
                    nc.sync.dma_start(
                        out=bq_sb, in_=bq[:].rearrange("(t p) -> p t", p=P)
                    )
                elif sc == 2:
                    prefetch_block(0)

            # V-projection tile: matmuls into a sps-tag PSUM strip so it
            # can interleave with block-0 pair-0 scores without touching
            # the ctx accumulators; PSUM drain on ScalarE.
            vh_heads = vh_sb[:, :, 0 : HL * VW].rearrange("p t (h c) -> p t h c", c=VW)

            def v_proj_tile(st):
                xv_t = xs.tile([P, KC, P], f16, name="xv_t")
                nc.sync.dma_start(
                    out=xv_t,
                    in_=xvT[:, st * P : (st + 1) * P].rearrange(
                        "(c p) j -> p c j", p=P
                    ),
                )
                psv = ps.tile([P, 1024], f32, name="psv", tag="sps", bufs=3)
                for kc in range(KC):
                    nc.tensor.matmul(
                        psv[:, 0:512],
                        lhsT=xv_t[:, kc, :],
                        rhs=wv_sb[:, kc, :],
                        start=(kc == 0),
                        stop=(kc == KC - 1),
                    )
                nc.scalar.copy(
                    vh_heads[:, st, :, 0:DK],
                    psv[:, 0:512].rearrange("p (h c) -> p h c", c=DK),
                )

            def q_proj_half(qc, half, on_scalar=False):
                """Half of a Q-projection chunk (2 dim-tiles); bias-add +
                PSUM drain on ScalarE (prologue) or VectorE (in blocks)."""
                sl = slice(qc * 512, (qc + 1) * 512)
                psq = ps.tile([P, 1024], f32, name=f"psq{qc}_{half}", tag="sps", bufs=3)
                for sub in range(2):
                    dt_ = 2 * half + sub
                    wslice = slice(dt_ * P, (dt_ + 1) * P)
                    hsl = slice(sub * 512, sub * 512 + 512)
                    for kc in range(KC):
                        nc.tensor.matmul(
                            psq[:, hsl],
                            lhsT=wq_sb[:, kc, wslice],
                            rhs=xq_tiles[qc][:, kc, :],
                            start=(kc == 0),
                            stop=(kc == KC - 1),
                        )
                for sub in range(2):
                    dt_ = 2 * half + sub
                    src = psq[:, sub * 512 : sub * 512 + 512]
                    if on_scalar:
                        nc.scalar.activation(
                            qhT_sb[:, dt_, sl],
                            src,
                            AF.Identity,
                            bias=bq_sb[:, dt_ : dt_ + 1],
                        )
                    else:
                        nc.vector.tensor_scalar_add(
                            out=qhT_sb[:, dt_, sl],
                            in0=src,
                            scalar1=bq_sb[:, dt_ : dt_ + 1],
                        )

            q_proj_half(0, 0, True)
            q_proj_half(0, 1, True)

            def o_proj2(qc, ot0):
                """Output projection for chunk qc, dim-tiles ot0 and
                ot0+1, in one sps-tag PSUM strip (self-contained so it
                can slot between attention tiles without touching the
                ctx accumulators).  No bias — host adds Wo@bv + bo."""
                qsl = slice(qc * 512, (qc + 1) * 512)
                pso = ps.tile([P, 1024], f32, name="pso", tag="sps", bufs=3)
                for half in range(2):
                    ot = ot0 + half
                    hsl = slice(half * 512, half * 512 + 512)
                    for cc in range(CC):
                        nc.tensor.matmul(
                            pso[:, hsl],
                            lhsT=wo_sb[:, cc, ot * P : (ot + 1) * P],
                            rhs=ctxT_sb[:, cc, qsl],
                            start=(cc == 0),
                            stop=(cc == CC - 1),
                        )
                for half in range(2):
                    ot = ot0 + half
                    o_sb = outp.tile([P, 512], f16, name="o_sb")
                    nc.vector.tensor_copy(
                        o_sb[:, :], pso[:, half * 512 : half * 512 + 512]
                    )
                    nc.sync.dma_start(
                        out=outT[ot * P : (ot + 1) * P, qsl], in_=o_sb[:, :]
                    )

            def attn_pair(qc, pair, with_vproj, fillers=None):
                """Scores -> masked exp -> P@V for one head pair.  In
                block 0 / pair 0 the V-projection tiles ride the same PE
                stream just-in-time ahead of the P@V consumers."""
                qsl = slice(qc * 512, (qc + 1) * 512)
                m_sb = m_tiles[qc]
                hA, hB = 2 * pair, 2 * pair + 1
                ctx_A = ps.tile([P, 512], f32, name="ctx_A", tag="ctxps", bufs=2)
                ctx_B = ps.tile([P, 512], f32, name="ctx_B", tag="ctxps", bufs=2)
                LAG = 2
                pend = {}
                for kt in range(ST + LAG):
                    if kt < ST:
                        ksl = slice(kt * P, (kt + 1) * P)
                        s_AB = ps.tile([P, 1024], f32, name="s_AB", tag="sps", bufs=3)
                        nc.tensor.matmul(
                            s_AB[:, 0:512],
                            lhsT=khT_sb[:, pair, 0, ksl],
                            rhs=qhT_sb[:, pair, qsl],
                        )
                        nc.tensor.matmul(
                            s_AB[:, 512:1024],
                            lhsT=khT_sb[:, pair, 1, ksl],
                            rhs=qhT_sb[:, pair, qsl],
                        )
                        p_AB = ptp.tile([P, 2, 512], f16, name="p_AB")
                        nc.scalar.activation(p_AB[:, :, :], s_AB[:, :].rearrange("p (h j) -> p h j", h=2), AF.Exp)
                        nc.vector.tensor_mul(
                            p_AB[:, :, :],
                            p_AB[:, :, :],
                            m_sb[:, kt, None, :].broadcast_to([P, 2, 512]),
                        )
                        pend[kt] = (p_AB[:, 0, :], p_AB[:, 1, :])
                        if with_vproj:
                            v_proj_tile(kt)
                            if kt == 4:
                                prefetch_xq(1)
                            elif kt == 8:
                                prefetch_mask(1)
                            elif kt == 12:
                                nc.sync.dma_start(
                                    out=wo_sb,
                                    in_=woT[:, :].rearrange("(c p) m -> p c m", p=P),
                                )
                    kv = kt - LAG
                    if kv >= 0:
                        q_A, q_B = pend.pop(kv)
                        nc.tensor.matmul(
                            ctx_A[:, :],
                            lhsT=vh_sb[:, kv, hA * VW : hA * VW + P],
                            rhs=q_A[:, :],
                            start=(kv == 0),
                            stop=(kv == ST - 1),
                        )
                        nc.tensor.matmul(
                            ctx_B[:, :],
                            lhsT=vh_sb[:, kv, hB * VW : hB * VW + P],
                            rhs=q_B[:, :],
                            start=(kv == 0),
                            stop=(kv == ST - 1),
                        )
                    if fillers and kt in (5, 10):
                        fillers.pop(0)()
                # normalization: recip of denominator rows, broadcast
                # across partitions on GpSimd, multiply on VectorE.
                # Head B's 64 rows then move to partitions 64-127 of
                # ctxT via a small SBUF->SBUF DMA (engines cannot
                # shift data across partitions).
                dens = small.tile([1, 2, 512], f32, name="dens")
                nc.vector.tensor_copy(dens[0:1, 0, :], ctx_A[DK : DK + 1, :])
                nc.vector.tensor_copy(dens[0:1, 1, :], ctx_B[DK : DK + 1, :])
                recips = small.tile([1, 2, 512], f32, name="recips")
                nc.vector.reciprocal_approx_fast(
                    out=recips[0:1, :, :], in_=dens[0:1, :, :]
                )
                bcA = small.tile([DK, 512], f32, name="bcA")
                bcB = small.tile([DK, 512], f32, name="bcB")
                nc.gpsimd.partition_broadcast(bcA[:, :], recips[0:1, 0, :])
                nc.gpsimd.partition_broadcast(bcB[:, :], recips[0:1, 1, :])
                nc.vector.tensor_mul(
                    ctxT_sb[0:DK, pair, qsl], ctx_A[0:DK, :], bcA[:, :]
                )
                stgB = small.tile([DK, 512], f16, name="stgB")
                nc.vector.tensor_mul(stgB[:, :], ctx_B[0:DK, :], bcB[:, :])
                nc.sync.dma_start(
                    out=ctxT_sb[DK : 2 * DK, pair, qsl], in_=stgB[:, :]
                )
                if fillers:
                    fillers.pop(0)()

            # ================= main pipeline =================
            # Q-proj of the next chunk and O-proj of the previous chunk
            # are sliced between the pairs so the PE absorbs them in the
            # slack of the ScalarE-paced attention stream — the exp
            # stream never starves at block boundaries.
            for qc in range(SC):
                if 1 <= qc < SC - 1:
                    prefetch_block(qc + 1)  # qc 0/1 prefetched earlier

                fillers = []
                if qc + 1 < SC:
                    fillers.append(lambda n=qc + 1: q_proj_half(n, 0))
                    fillers.append(lambda n=qc + 1: q_proj_half(n, 1))
                if qc > 0:
                    for ot0 in range(0, OT, 2):
                        fillers.append(lambda n=qc - 1, o=ot0: o_proj2(n, o))

                attn_pair(qc, 0, with_vproj=(qc == 0),
                          fillers=(None if qc == 0 else fillers))
                for pair in range(1, PAIRS):
                    attn_pair(qc, pair, with_vproj=False, fillers=fillers)
                while fillers:
                    fillers.pop(0)()
                m_tiles.pop(qc)

            for ot0 in range(0, OT, 2):
                o_proj2(SC - 1, ot0)

    nc.finalize()
    return nc


def prepare(q, k, v, mask, Wq, bq, Wk, bk, Wv, bv, Wo, bo):
    """Build the 8 per-core input maps + the exact host-side bias fold."""
    scale = np.float32(1.0 / np.sqrt(DK))
    in_maps = []
    per_batch = {}
    for b in range(B):
        per_batch[b] = dict(
            xqT=np.ascontiguousarray(q[b].T).astype(np.float16),
            xkT=np.ascontiguousarray(k[b].T).astype(np.float16),
            xvT=np.ascontiguousarray(v[b].T).astype(np.float16),
            maskT=np.ascontiguousarray(mask[b].T).astype(np.float16),
        )
    for c in range(N_CORES):
        b, g = c // 2, c % 2
        hsl = slice(g * DH, (g + 1) * DH)
        in_maps.append(
            dict(
                per_batch[b],
                wqT=np.ascontiguousarray((Wq[hsl, :] * scale).T).astype(np.float16),
                wkT=np.ascontiguousarray(Wk[hsl, :].T).astype(np.float16),
                wvT=np.ascontiguousarray(Wv[hsl, :].T).astype(np.float16),
                woT=np.ascontiguousarray(Wo[:, hsl].T).astype(np.float16),
                bq=np.ascontiguousarray(bq[hsl] * scale, dtype=np.float32),
            )
        )
    # softmax rows sum to 1 => ctx = ctx' + bv; out = ctx' Wo^T + (Wo bv + bo)
    host_bias = (bo.astype(np.float64) + Wo.astype(np.float64) @ bv.astype(np.float64)).astype(np.float32)
    return in_maps, host_bias


def finish(res, host_bias):
    out = np.empty((B, S, D), dtype=np.float32)
    for b in range(B):
        partial = res.results[2 * b]["outT"].astype(np.float32) + res.results[
            2 * b + 1
        ]["outT"].astype(np.float32)
        out[b] = partial.T + host_bias
    return out


def run_on_cores(in_maps, trace=False):
    global _compiled
    from concourse import bass_utils

    if _compiled is None:
        _compiled = _build_program()
    res = bass_utils.run_bass_kernel_spmd(
        _compiled, in_maps, core_ids=list(range(N_CORES)), trace=trace
    )
    return res


def kernel(q, k, v, mask, Wq, bq, Wk, bk, Wv, bv, Wo, bo):
    in_maps, host_bias = prepare(q, k, v, mask, Wq, bq, Wk, bk, Wv, bv, Wo, bo)
    res = run_on_cores(in_maps)
    return finish(res, host_bias)


# revision 24
# speedup vs baseline: 1.0243x; 1.0243x over previous
"""Multi-head attention kernel for 8 Trainium2 NeuronCores.

Problem: B=4, S=2048, D=1024, H=16 heads (d_k=64), fp32 inputs,
random 0/1 attention mask [B, S, S].

Sharding: core c -> (batch b = c//2, head-group g = c%2).  Each core
computes 8 heads of one batch: Megatron column-parallel QKV, row-parallel
output projection.  Host sums the two partial outputs per batch.

Pipeline layout (single pass, engines overlapped):
  prologue: K-proj (all keys), V-proj (all keys), Q-proj(chunk 0)
  block qc=0..3: attention(qc) | Q-proj(qc+1) | O-proj(qc)
so the ScalarE exp stream (the second-largest engine load) hides under
the PE stream instead of serializing behind a monolithic phase 1.

Bias algebra (exact):
  - bk dropped: s[k,q] += qh[q].bk is constant over k at fixed q and
    softmax over k is shift-invariant.
  - bv, bo folded to host: softmax rows sum to 1, so ctx = ctx' + bv
    and out = ctx' Wo^T + (Wo bv + bo).  Host adds the single vector.
  - bq kept on device (varies over k), scale 1/sqrt(dk) folded into Wq.

Device-side layout choices (avoids every on-device transpose):
  - host passes x^T [D, S] so projections contract D on partitions
  - projections emit qh^T / kh^T [512, S] (head dims on partitions)
  - scores are computed transposed: S^T[k, q] = kh^T.T @ qh^T
  - softmax: exp on ScalarE (no max subtraction; scores are O(5)),
    multiplicative fp16 {0,1} mask on VectorE (2x packed mode),
    denominator = ones-column appended to V in the P@V matmul
  - ctx^T[d, q] accumulates in PSUM; normalization multiplies by a
    reciprocal row broadcast across partitions via GpSimd
  - output projection consumes ctx^T directly, emits fp16 out^T partials

All matmuls keep the stock (128,128) PE tile shape — K=64 / M=65
variants measurably drop the PE instruction stream out of its fast
decode path on HW (+70ns per matmul).  vh stores 65 real columns per
head (64 v-dims + ones) in a 584-wide strip; each head's stationary
read spans 128 columns, the 63-column overlap into the next head only
lands in PSUM rows 65..127 which are never read.
"""

import numpy as np

B = 4
S = 2048
D = 1024
H = 16  # total heads
HL = 8  # heads per core
DK = 64
DH = HL * DK  # 512 local head dims
P = 128
N_CORES = 8

_compiled = None


def _build_program():
    import concourse.bacc as bacc
    import concourse.tile as tile
    from concourse import mybir

    f32 = mybir.dt.float32
    f16 = mybir.dt.float16
    AF = mybir.ActivationFunctionType

    nc = bacc.Bacc()

    # ---- DRAM I/O ----
    xqT = nc.declare_dram_parameter("xqT", [D, S], f16, isOutput=False)
    xkT = nc.declare_dram_parameter("xkT", [D, S], f16, isOutput=False)
    xvT = nc.declare_dram_parameter("xvT", [D, S], f16, isOutput=False)
    maskT = nc.declare_dram_parameter("maskT", [S, S], f16, isOutput=False)
    wqT = nc.declare_dram_parameter("wqT", [D, DH], f16, isOutput=False)
    wkT = nc.declare_dram_parameter("wkT", [D, DH], f16, isOutput=False)
    wvT = nc.declare_dram_parameter("wvT", [D, DH], f16, isOutput=False)
    woT = nc.declare_dram_parameter("woT", [DH, D], f16, isOutput=False)
    bq = nc.declare_dram_parameter("bq", [DH], f32, isOutput=False)
    outT = nc.declare_dram_parameter("outT", [D, S], f16, isOutput=True)

    KC = D // P       # 8 contraction chunks for QKV projections
    DT = DH // P      # 4 dim-tiles of qh^T/kh^T
    SC = S // 512     # 4 seq chunks of 512
    ST = S // P       # 16 seq tiles of 128
    OT = D // P       # 8 output dim tiles
    CC = DH // P      # 4 contraction chunks for O-projection
    PAIRS = HL // 2   # 4 head pairs
    VW = DK + 1       # 65 stored columns per head in vh (v-dims + ones)
    VPAD = HL * VW + P - VW  # 583 -> stationary reads stay in bounds

    with tile.TileContext(nc) as tc:
        with (
            tc.tile_pool(name="persist", bufs=1) as persist,
            tc.tile_pool(name="maskp", bufs=2) as maskp,
            tc.tile_pool(name="xs", bufs=2) as xs,
            tc.tile_pool(name="pt", bufs=3) as ptp,
            tc.tile_pool(name="small", bufs=2) as small,
            tc.tile_pool(name="outp", bufs=2) as outp,
            tc.tile_pool(name="ps", bufs=2, space="PSUM") as ps,
        ):
            qhT_sb = persist.tile([P, DT, S], f16)
            khT_sb = persist.tile([P, PAIRS, 2, S], f16)
            vh_sb = persist.tile([P, ST, VPAD + 1], f16)
            ctxT_sb = persist.tile([P, CC, S], f16)
            wq_sb = persist.tile([P, KC, DH], f16)
            wk_sb = persist.tile([P, KC, DH], f16)
            wv_sb = persist.tile([P, KC, DH], f16)
            wo_sb = persist.tile([P, CC, D], f16)
            bq_sb = persist.tile([P, DT], f32)

            def load_w(dst, src):
                nc.sync.dma_start(
                    out=dst, in_=src[:, :].rearrange("(c p) m -> p c m", p=P)
                )

            # K-proj weights + first xk chunk lead the DMA queue; the
            # rest of the bulk is staggered between the per-sc loads so
            # no xk chunk queues behind multi-MB transfers it doesn't need.
            load_w(wk_sb, wkT)
            xk_tiles = {}
            xk_tiles[0] = xs.tile([P, KC, 512], f16, name="xk_t")
            nc.sync.dma_start(
                out=xk_tiles[0], in_=xkT[:, 0:512].rearrange("(c p) j -> p c j", p=P)
            )

            # zero-init padded K layout; ones columns of vh give softmax
            # denominators.  Zero kh rows keep the full-K matmul exact.
            nc.vector.memset(khT_sb[:, :, :, :], 0.0)
            nc.vector.memset(
                vh_sb[:, :, 0 : HL * VW].rearrange(
                    "p t (h c) -> p t h c", c=VW
                )[:, :, :, DK : DK + 1],
                1.0,
            )
            # the 64-col tail is read (never written) by head 7's
            # 128-wide stationary fetch — keep it NaN-free
            nc.vector.memset(vh_sb[:, :, HL * VW :], 0.0)

            m_tiles = {}
            xq_tiles = {}

            def prefetch_mask(qc):
                m_tiles[qc] = maskp.tile([P, ST, 512], f16, name="m_sb")
                nc.sync.dma_start(
                    out=m_tiles[qc],
                    in_=maskT[:, qc * 512 : (qc + 1) * 512].rearrange(
                        "(t p) j -> p t j", p=P
                    ),
                )

            def prefetch_xq(qc):
                xq_tiles[qc] = xs.tile([P, KC, 512], f16, name="xk_t")
                nc.sync.dma_start(
                    out=xq_tiles[qc],
                    in_=xqT[:, qc * 512 : (qc + 1) * 512].rearrange(
                        "(c p) j -> p c j", p=P
                    ),
                )

            def prefetch_block(qc):
                prefetch_mask(qc)
                prefetch_xq(qc)

            # ================= prologue =================
            # K-projection: all 4 seq chunks -> khT (copies on ScalarE,
            # idle here; bk dropped exactly — softmax shift-invariance)
            for sc in range(SC):
                sl = slice(sc * 512, (sc + 1) * 512)
                xk_t = xk_tiles.pop(sc)
                if sc + 1 < SC:
                    xk_tiles[sc + 1] = xs.tile([P, KC, 512], f16, name="xk_t")
                    nc.sync.dma_start(
                        out=xk_tiles[sc + 1],
                        in_=xkT[:, (sc + 1) * 512 : (sc + 2) * 512].rearrange(
                            "(c p) j -> p c j", p=P
                        ),
                    )
                for half in range(2):
                    psk = ps.tile([P, 1024], f32, name=f"psk{sc}_{half}", tag="sps", bufs=2)
                    for sub in range(2):
                        dt_ = 2 * half + sub
                        wslice = slice(dt_ * P, (dt_ + 1) * P)
                        hsl = slice(sub * 512, sub * 512 + 512)
                        for kc in range(KC):
                            nc.tensor.matmul(
                                psk[:, hsl],
                                lhsT=wk_sb[:, kc, wslice],
                                rhs=xk_t[:, kc, :],
                                start=(kc == 0),
                                stop=(kc == KC - 1),
                            )
                    for sub in range(2):
                        dt_ = 2 * half + sub
                        src = psk[:, sub * 512 : sub * 512 + 512]
                        nc.scalar.copy(khT_sb[0:DK, dt_, 0, sl], src[0:DK, :])
                        nc.scalar.copy(khT_sb[DK : 2 * DK, dt_, 1, sl], src[DK : 2 * DK, :])
                # staggered bulk DMA: each group rides behind the xk
                # chunk it must not delay
                if sc == 0:
                    load_w(wv_sb, wvT)
                elif sc == 1:
                    load_w(wq_sb, wqT)
                    nc.sync.dma_start(
                        out=bq_sb, in_=bq[:].rearrange("(t p) -> p t", p=P)
                    )
                elif sc == 2:
                    prefetch_block(0)

            # V-projection tile: matmuls into a sps-tag PSUM strip so it
            # can interleave with block-0 pair-0 scores without touching
            # the ctx accumulators; PSUM drain on ScalarE.
            vh_heads = vh_sb[:, :, 0 : HL * VW].rearrange("p t (h c) -> p t h c", c=VW)

            def v_proj_tile(st):
                xv_t = xs.tile([P, KC, P], f16, name="xv_t")
                nc.sync.dma_start(
                    out=xv_t,
                    in_=xvT[:, st * P : (st + 1) * P].rearrange(
                        "(c p) j -> p c j", p=P
                    ),
                )
                psv = ps.tile([P, 1024], f32, name="psv", tag="sps", bufs=2)
                for kc in range(KC):
                    nc.tensor.matmul(
                        psv[:, 0:512],
                        lhsT=xv_t[:, kc, :],
                        rhs=wv_sb[:, kc, :],
                        start=(kc == 0),
                        stop=(kc == KC - 1),
                    )
                nc.scalar.copy(
                    vh_heads[:, st, :, 0:DK],
                    psv[:, 0:512].rearrange("p (h c) -> p h c", c=DK),
                )

            def q_proj_quarter(qc, dt_, on_scalar=False):
                """One dim-tile of a Q-projection chunk — a self-contained
                8-matmul PSUM strip small enough to slot between attention
                tiles without starving the exp stream; bias-add + PSUM
                drain on ScalarE (prologue) or VectorE (in blocks)."""
                sl = slice(qc * 512, (qc + 1) * 512)
                wslice = slice(dt_ * P, (dt_ + 1) * P)
                psq = ps.tile([P, 1024], f32, name=f"psq{qc}_{dt_}", tag="sps", bufs=2)
                for kc in range(KC):
                    nc.tensor.matmul(
                        psq[:, 0:512],
                        lhsT=wq_sb[:, kc, wslice],
                        rhs=xq_tiles[qc][:, kc, :],
                        start=(kc == 0),
                        stop=(kc == KC - 1),
                    )
                src = psq[:, 0:512]
                if on_scalar:
                    nc.scalar.activation(
                        qhT_sb[:, dt_, sl],
                        src,
                        AF.Identity,
                        bias=bq_sb[:, dt_ : dt_ + 1],
                    )
                else:
                    nc.vector.tensor_scalar_add(
                        out=qhT_sb[:, dt_, sl],
                        in0=src,
                        scalar1=bq_sb[:, dt_ : dt_ + 1],
                    )

            for dt_ in range(DT):
                q_proj_quarter(0, dt_, True)

            def o_proj2(qc, ot0):
                """Output projection for chunk qc, dim-tiles ot0 and
                ot0+1, in one sps-tag PSUM strip (self-contained so it
                can slot between attention tiles without touching the
                ctx accumulators).  No bias — host adds Wo@bv + bo."""
                qsl = slice(qc * 512, (qc + 1) * 512)
                pso = ps.tile([P, 1024], f32, name="pso", tag="sps", bufs=2)
                for half in range(2):
                    ot = ot0 + half
                    hsl = slice(half * 512, half * 512 + 512)
                    for cc in range(CC):
                        nc.tensor.matmul(
                            pso[:, hsl],
                            lhsT=wo_sb[:, cc, ot * P : (ot + 1) * P],
                            rhs=ctxT_sb[:, cc, qsl],
                            start=(cc == 0),
                            stop=(cc == CC - 1),
                        )
                for half in range(2):
                    ot = ot0 + half
                    o_sb = outp.tile([P, 512], f16, name="o_sb")
                    nc.vector.tensor_copy(
                        o_sb[:, :], pso[:, half * 512 : half * 512 + 512]
                    )
                    nc.sync.dma_start(
                        out=outT[ot * P : (ot + 1) * P, qsl], in_=o_sb[:, :]
                    )

            def attn_pair(qc, pair, with_vproj, fillers=None):
                """Scores -> masked exp -> P@V for one head pair.  In
                block 0 / pair 0 the V-projection tiles ride the same PE
                stream just-in-time ahead of the P@V consumers."""
                qsl = slice(qc * 512, (qc + 1) * 512)
                m_sb = m_tiles[qc]
                hA, hB = 2 * pair, 2 * pair + 1
                ctx_A = ps.tile([P, 512], f32, name="ctx_A", tag="ctxps", bufs=4)
                ctx_B = ps.tile([P, 512], f32, name="ctx_B", tag="ctxps", bufs=4)
                LAG = 2
                pend = {}
                for kt in range(ST + LAG):
                    if kt < ST:
                        ksl = slice(kt * P, (kt + 1) * P)
                        s_AB = ps.tile([P, 1024], f32, name="s_AB", tag="sps", bufs=2)
                        nc.tensor.matmul(
                            s_AB[:, 0:512],
                            lhsT=khT_sb[:, pair, 0, ksl],
                            rhs=qhT_sb[:, pair, qsl],
                        )
                        nc.tensor.matmul(
                            s_AB[:, 512:1024],
                            lhsT=khT_sb[:, pair, 1, ksl],
                            rhs=qhT_sb[:, pair, qsl],
                        )
                        p_AB = ptp.tile([P, 2, 512], f16, name="p_AB")
                        nc.scalar.activation(p_AB[:, :, :], s_AB[:, :].rearrange("p (h j) -> p h j", h=2), AF.Exp)
                        nc.vector.tensor_mul(
                            p_AB[:, :, :],
                            p_AB[:, :, :],
                            m_sb[:, kt, None, :].broadcast_to([P, 2, 512]),
                        )
                        pend[kt] = (p_AB[:, 0, :], p_AB[:, 1, :])
                        if with_vproj:
                            v_proj_tile(kt)
                            if kt == 4:
                                prefetch_xq(1)
                            elif kt == 8:
                                prefetch_mask(1)
                            elif kt == 12:
                                nc.sync.dma_start(
                                    out=wo_sb,
                                    in_=woT[:, :].rearrange("(c p) m -> p c m", p=P),
                                )
                    kv = kt - LAG
                    if kv >= 0:
                        q_A, q_B = pend.pop(kv)
                        nc.tensor.matmul(
                            ctx_A[:, :],
                            lhsT=vh_sb[:, kv, hA * VW : hA * VW + P],
                            rhs=q_A[:, :],
                            start=(kv == 0),
                            stop=(kv == ST - 1),
                        )
                        nc.tensor.matmul(
                            ctx_B[:, :],
                            lhsT=vh_sb[:, kv, hB * VW : hB * VW + P],
                            rhs=q_B[:, :],
                            start=(kv == 0),
                            stop=(kv == ST - 1),
                        )
                    if fillers and kt in (5, 10):
                        fillers.pop(0)()
                # normalization: recip of denominator rows, broadcast
                # across partitions on GpSimd, multiply on VectorE.
                # Head B's 64 rows then move to partitions 64-127 of
                # ctxT via a small SBUF->SBUF DMA (engines cannot
                # shift data across partitions).
                dens = small.tile([1, 2, 512], f32, name="dens")
                nc.vector.tensor_copy(dens[0:1, 0, :], ctx_A[DK : DK + 1, :])
                nc.vector.tensor_copy(dens[0:1, 1, :], ctx_B[DK : DK + 1, :])
                recips = small.tile([1, 2, 512], f32, name="recips")
                nc.vector.reciprocal_approx_fast(
                    out=recips[0:1, :, :], in_=dens[0:1, :, :]
                )
                bcA = small.tile([DK, 512], f32, name="bcA")
                bcB = small.tile([DK, 512], f32, name="bcB")
                nc.gpsimd.partition_broadcast(bcA[:, :], recips[0:1, 0, :])
                nc.gpsimd.partition_broadcast(bcB[:, :], recips[0:1, 1, :])
                nc.vector.tensor_mul(
                    ctxT_sb[0:DK, pair, qsl], ctx_A[0:DK, :], bcA[:, :]
                )
                stgB = small.tile([DK, 512], f16, name="stgB")
                nc.vector.tensor_mul(stgB[:, :], ctx_B[0:DK, :], bcB[:, :])
                nc.sync.dma_start(
                    out=ctxT_sb[DK : 2 * DK, pair, qsl], in_=stgB[:, :]
                )
                if fillers:
                    fillers.pop(0)()

            # ================= main pipeline =================
            # Q-proj of the next chunk and O-proj of the previous chunk
            # are sliced between the pairs so the PE absorbs them in the
            # slack of the ScalarE-paced attention stream — the exp
            # stream never starves at block boundaries.
            for qc in range(SC):
                if 1 <= qc < SC - 1:
                    prefetch_block(qc + 1)  # qc 0/1 prefetched earlier

                fillers = []
                if qc + 1 < SC:
                    for dt_ in range(DT):
                        fillers.append(lambda n=qc + 1, t=dt_: q_proj_quarter(n, t))
                if qc > 0:
                    for ot0 in range(0, OT, 2):
                        fillers.append(lambda n=qc - 1, o=ot0: o_proj2(n, o))

                attn_pair(qc, 0, with_vproj=(qc == 0),
                          fillers=(None if qc == 0 else fillers))
                for pair in range(1, PAIRS):
                    attn_pair(qc, pair, with_vproj=False, fillers=fillers)
                while fillers:
                    fillers.pop(0)()
                m_tiles.pop(qc)

            for ot0 in range(0, OT, 2):
                o_proj2(SC - 1, ot0)

    nc.finalize()
    return nc


def prepare(q, k, v, mask, Wq, bq, Wk, bk, Wv, bv, Wo, bo):
    """Build the 8 per-core input maps + the exact host-side bias fold."""
    scale = np.float32(1.0 / np.sqrt(DK))
    in_maps = []
    per_batch = {}
    for b in range(B):
        per_batch[b] = dict(
            xqT=np.ascontiguousarray(q[b].T).astype(np.float16),
            xkT=np.ascontiguousarray(k[b].T).astype(np.float16),
            xvT=np.ascontiguousarray(v[b].T).astype(np.float16),
            maskT=np.ascontiguousarray(mask[b].T).astype(np.float16),
        )
    for c in range(N_CORES):
        b, g = c // 2, c % 2
        hsl = slice(g * DH, (g + 1) * DH)
        in_maps.append(
            dict(
                per_batch[b],
                wqT=np.ascontiguousarray((Wq[hsl, :] * scale).T).astype(np.float16),
                wkT=np.ascontiguousarray(Wk[hsl, :].T).astype(np.float16),
                wvT=np.ascontiguousarray(Wv[hsl, :].T).astype(np.float16),
                woT=np.ascontiguousarray(Wo[:, hsl].T).astype(np.float16),
                bq=np.ascontiguousarray(bq[hsl] * scale, dtype=np.float32),
            )
        )
    # softmax rows sum to 1 => ctx = ctx' + bv; out = ctx' Wo^T + (Wo bv + bo)
    host_bias = (bo.astype(np.float64) + Wo.astype(np.float64) @ bv.astype(np.float64)).astype(np.float32)
    return in_maps, host_bias


def finish(res, host_bias):
    out = np.empty((B, S, D), dtype=np.float32)
    for b in range(B):
        partial = res.results[2 * b]["outT"].astype(np.float32) + res.results[
            2 * b + 1
        ]["outT"].astype(np.float32)
        out[b] = partial.T + host_bias
    return out


def run_on_cores(in_maps, trace=False):
    global _compiled
    from concourse import bass_utils

    if _compiled is None:
        _compiled = _build_program()
    res = bass_utils.run_bass_kernel_spmd(
        _compiled, in_maps, core_ids=list(range(N_CORES)), trace=trace
    )
    return res


def kernel(q, k, v, mask, Wq, bq, Wk, bk, Wv, bv, Wo, bo):
    in_maps, host_bias = prepare(q, k, v, mask, Wq, bq, Wk, bk, Wv, bv, Wo, bo)
    res = run_on_cores(in_maps)
    return finish(res, host_bias)
